# revision 15
# baseline (speedup 1.0000x reference)
"""DGCNN two-branch forward pass on 8 Trainium2 NeuronCores.

Sharding: pure data parallel over batch B=16 -> 2 samples per core,
replicated weights.  Inside each core the two samples are processed in
lockstep (paired into 128-partition tiles where possible).

Pipeline per edge-conv unit (4 units: TNet-edge, EC0, EC1, EC2):
  1. augmented matmul produces per-row-order-equivalent distances
     d[n,m] = <x_n,x_m> - |x_m|^2/2  (row-constant dropped; exact fp32)
  2. exact top-24 per row via DVE max8/max_index/match_replace (3 rounds)
  3. index lists rewritten into the 16-partition-wrapped int16 layout the
     GPSIMD ap_gather ucode expects, via 16-wide PE transposes (on chip)
  4. ap_gather pulls neighbor features channel-major (no transposes)
  5. h1 = relu(gather(A) + P) with A = W2 @ x, P = (W1-W2) @ x + b
     (first 1x1 conv folded into per-point transforms)
  6. conv2 matmul + grouped max over k + relu -> next x
Heads (tn_loc/FCs/trans, loc/global FCs, seg stack) are plain matmuls.
"""

import os
import numpy as np

B = 16
NCORES = 8
SPC = B // NCORES      # samples per core
K = 20                 # neighbors
KP = 32                # padded neighbor slots (for the 16-wrap)
N_FULL = 2048

_BUILD_CACHE = {}


def _build(N):
    """Build + compile the per-core Bass module for point count N."""
    if N in _BUILD_CACHE:
        return _BUILD_CACHE[N]

    from contextlib import ExitStack
    import concourse.bacc as bacc
    import concourse.tile as tile
    import concourse.mybir as mybir
    from concourse.bass_types import AP

    F32 = mybir.dt.float32
    I16 = mybir.dt.int16
    U32 = mybir.dt.uint32
    AF = mybir.ActivationFunctionType
    OP = mybir.AluOpType
    AX = mybir.AxisListType

    T = N // 128           # row tiles
    CB = min(512, N)       # matmul column block
    NCB = N // CB
    UC = [3, 3, 64, 64]    # input channels per edge unit
    WROW = T * 256         # wrapped idx row length per sample

    nc = bacc.Bacc("TRN2", target_bir_lowering=False, debug=False,
                   enable_asserts=False, num_devices=NCORES)
    dt = nc.dram_tensor

    def din(name, shape):
        return dt(name, shape, F32, kind="ExternalInput").ap()

    pts = din("pts", (SPC, 3, N))
    waT = [din(f"waT{k}", (UC[k] + 1, 64)) for k in range(4)]
    wpT = [din(f"wpT{k}", (UC[k] + 1, 64)) for k in range(4)]
    c2T0 = din("c2T0", (128, 128))
    c2b0 = din("c2b0", (128, 1))
    blkT = [None] + [din(f"blkT{k}", (128, 128)) for k in (1, 2, 3)]
    blkb = [None] + [din(f"blkb{k}", (128, 1)) for k in (1, 2, 3)]
    ones_d = din("ones", (64, 1))
    eye_d = din("eye128", (128, 128))
    neg1_d = din("neg1", (1, N))
    tn_locT = din("tn_locT", (128, 1024))
    tn_loc_b = din("tn_loc_b", (128, 8))
    tn_g0T = din("tn_g0T", (1024, 512))
    tn_g0_b = din("tn_g0_b", (1, 512))
    tn_g1T = din("tn_g1T", (512, 256))
    tn_g1_b = din("tn_g1_b", (1, 256))
    tn_linT = din("tn_linT", (256, 9))
    tn_lin_be = din("tn_lin_be", (1, 9))
    locTa = din("locTa", (128, 1024))
    locTb = din("locTb", (128, 1024))
    loc_b = din("loc_b", (128, 8))
    g0T = din("g0T", (1024, 256))
    g0_b = din("g0_b", (1, 256))
    g1T = din("g1T", (256, 256))
    g1_b = din("g1_b", (1, 256))
    g2T = din("g2T", (256, 128))
    g2_b = din("g2_b", (1, 128))
    goT = din("goT", (128, 16))
    go_b = din("go_b", (1, 16))
    seg0AT = din("seg0AT", (1024, 256))
    seg0BTa = din("seg0BTa", (128, 256))
    seg0BTb = din("seg0BTb", (128, 256))
    seg0_bp = din("seg0_bp", (128, 2))
    seg1T = din("seg1T", (256, 256))
    seg1_bp = din("seg1_bp", (128, 2))
    csT = din("csT", (256, 128))
    cs_bp = din("cs_bp", (128, 1))
    moT = din("moT", (128, 50))
    mo_b = din("mo_b", (50, 1))

    mask_out = dt("mask_out", (SPC, 50, N), F32, kind="ExternalOutput").ap()
    global_out = dt("global_out", (SPC, 16), F32, kind="ExternalOutput").ap()
    trans_out = dt("trans_out", (SPC, 9), F32, kind="ExternalOutput").ap()

    rt = dt("rt", (SPC, 1024), F32, kind="Internal").ap()

    with tile.TileContext(nc) as tc, ExitStack() as ctx:
        wts = ctx.enter_context(tc.tile_pool(name="wts", bufs=1))
        pw = ctx.enter_context(tc.tile_pool(name="work", bufs=1))
        pG = ctx.enter_context(tc.tile_pool(name="gpool", bufs=2))
        pX = ctx.enter_context(tc.tile_pool(name="xpool", bufs=1))
        pH = ctx.enter_context(tc.tile_pool(name="hpool", bufs=2))
        small = ctx.enter_context(tc.tile_pool(name="small", bufs=4))
        psA = ctx.enter_context(tc.tile_pool(name="psA", bufs=3, space="PSUM"))
        psC = ctx.enter_context(tc.tile_pool(name="psC", bufs=1, space="PSUM"))

        def load2d(ap, tag):
            sh = list(ap.shape)
            t = wts.tile(sh, F32, tag=tag, name=tag)
            nc.sync.dma_start(out=t[:], in_=ap)
            return t

        def load_chunked(ap, tag):
            # (c*128, U) dram -> (128, c*U) sbuf, [p, c*U + u] = ap[c*128+p, u]
            R, U = ap.shape
            c = R // 128
            t = wts.tile([128, c * U], F32, tag=tag, name=tag)
            nc.sync.dma_start(out=t[:].rearrange("p (c u) -> p c u", c=c),
                              in_=ap.rearrange("(c p) u -> p c u", p=128))
            return t

        waT_sb = [load2d(waT[k], f"waT{k}") for k in range(4)]
        wpT_sb = [load2d(wpT[k], f"wpT{k}") for k in range(4)]
        c2T0_sb = load2d(c2T0, "c2T0")
        c2b0_sb = load2d(c2b0, "c2b0")
        blkT_sb = [None] + [load2d(blkT[k], f"blkT{k}") for k in (1, 2, 3)]
        blkb_sb = [None] + [load2d(blkb[k], f"blkb{k}") for k in (1, 2, 3)]
        ones_sb = load2d(ones_d, "ones")
        eye_sb = load2d(eye_d, "eye128")
        tn_locT_sb = load2d(tn_locT, "tn_locT")
        tn_loc_b_sb = load2d(tn_loc_b, "tn_loc_b")
        tn_g0_b_sb = load2d(tn_g0_b, "tn_g0_b")
        tn_g1T_sb = load_chunked(tn_g1T, "tn_g1T")
        tn_g1_b_sb = load2d(tn_g1_b, "tn_g1_b")
        tn_linT_sb = load_chunked(tn_linT, "tn_linT")
        tn_lin_be_sb = load2d(tn_lin_be, "tn_lin_be")
        locTa_sb = load2d(locTa, "locTa")
        locTb_sb = load2d(locTb, "locTb")
        loc_b_sb = load2d(loc_b, "loc_b")
        g0_b_sb = load2d(g0_b, "g0_b")
        g1T_sb = load_chunked(g1T, "g1T")
        g1_b_sb = load2d(g1_b, "g1_b")
        g2T_sb = load_chunked(g2T, "g2T")
        g2_b_sb = load2d(g2_b, "g2_b")
        goT_sb = load2d(goT, "goT")
        go_b_sb = load2d(go_b, "go_b")
        seg0BTa_sb = load2d(seg0BTa, "seg0BTa")
        seg0BTb_sb = load2d(seg0BTb, "seg0BTb")
        seg0_bp_sb = load2d(seg0_bp, "seg0_bp")
        seg1T_sb = load_chunked(seg1T, "seg1T")
        seg1_bp_sb = load2d(seg1_bp, "seg1_bp")
        csT_sb = load_chunked(csT, "csT")
        cs_bp_sb = load2d(cs_bp, "cs_bp")
        moT_sb = load2d(moT, "moT")
        mo_b_sb = load2d(mo_b, "mo_b")

        # unit outputs: X12[s] also serves as the TNet edge output (overwritten
        # by units 1/2 after tn_loc consumed it)
        X12 = [pX.tile([128, N], F32, tag=f"X12_{s}", name=f"X12_{s}")
               for s in range(SPC)]
        X3 = pX.tile([128, N], F32, tag="X3", name="X3")

        def fill_sq(augR, src, C):
            """augR[C,:] = 0.5 * sum_c src[c,:]^2   (src: (C, N) sbuf AP)"""
            xx = pw.tile([64, N], F32, tag="xx", name="xx")
            nc.vector.tensor_tensor(out=xx[0:C, :], in0=src, in1=src, op=OP.mult)
            for q in range(NCB):
                ps = psA.tile([1, CB], F32, tag="a", name="psq")
                nc.tensor.matmul(out=ps[:], lhsT=ones_sb[0:C, 0:1],
                                 rhs=xx[0:C, q * CB:(q + 1) * CB],
                                 start=True, stop=True)
                nc.scalar.activation(out=xx[0:1, q * CB:(q + 1) * CB],
                                     in_=ps[:], func=AF.Copy, scale=0.5)
            nc.sync.dma_start(out=augR[C:C + 1, :], in_=xx[0:1, :])

        def new_aug(s, C):
            # rows 0:C = x, row C = -1 (lhsT) / 0.5*|x|^2 (rhs)
            aL = pw.tile([66, N], F32, tag=f"augL{s}", name=f"augL{s}")
            aR = pw.tile([66, N], F32, tag=f"augR{s}", name=f"augR{s}")
            nc.sync.dma_start(out=aL[C:C + 1, :], in_=neg1_d)
            return aL, aR

        # ---- unit 0 aug from raw points ----
        augL = [None, None]
        augR = [None, None]
        for s in range(SPC):
            aL, aR = new_aug(s, 3)
            nc.sync.dma_start(out=aR[0:3, :], in_=pts[s])
            nc.scalar.activation(out=aL[0:3, :], in_=aR[0:3, :], func=AF.Copy)
            fill_sq(aR, aR[0:3, :], 3)
            augL[s], augR[s] = aL, aR

        JBS = [(0, 8), (8, 16), (16, 20)]

        def edge_unit(k, augL, augR):
            C = UC[k]
            C1 = C + 1
            # A = W2 @ x ; P = (W1 - W2) @ x + b   (both (64, N), 2 samples stacked)
            A_t = pw.tile([128, N], F32, tag="A", name="A_t")
            P_t = pw.tile([128, N], F32, tag="P", name="P_t")
            for s in range(SPC):
                for q in range(NCB):
                    ps = psA.tile([64, CB], F32, tag="a", name="psa")
                    nc.tensor.matmul(out=ps[:], lhsT=waT_sb[k][:],
                                     rhs=augL[s][0:C1, q * CB:(q + 1) * CB],
                                     start=True, stop=True)
                    nc.scalar.activation(out=A_t[s * 64:(s + 1) * 64, q * CB:(q + 1) * CB],
                                         in_=ps[:], func=AF.Copy)
                    ps2 = psA.tile([64, CB], F32, tag="a", name="psp")
                    nc.tensor.matmul(out=ps2[:], lhsT=wpT_sb[k][:],
                                     rhs=augL[s][0:C1, q * CB:(q + 1) * CB],
                                     start=True, stop=True)
                    nc.scalar.activation(out=P_t[s * 64:(s + 1) * 64, q * CB:(q + 1) * CB],
                                         in_=ps2[:], func=AF.Copy)

            # distances + top-24 per row
            IDXs = []
            for s in range(SPC):
                IDX = pw.tile([128, T * 24], U32, tag=f"IDX{s}", name=f"IDX{s}")
                for t in range(T):
                    D = pw.tile([128, N], F32, tag="D", name="D")
                    for q in range(NCB):
                        ps = psA.tile([128, CB], F32, tag="a", name="psd")
                        nc.tensor.matmul(out=ps[:],
                                         lhsT=augL[s][0:C1, t * 128:(t + 1) * 128],
                                         rhs=augR[s][0:C1, q * CB:(q + 1) * CB],
                                         start=True, stop=True)
                        nc.scalar.activation(out=D[:, q * CB:(q + 1) * CB],
                                             in_=ps[:], func=AF.Copy)
                    for r in range(3):
                        v8 = small.tile([128, 8], F32, tag="v8", name="v8")
                        nc.vector.max(out=v8[:], in_=D[:])
                        nc.vector.max_index(
                            out=IDX[:, t * 24 + r * 8: t * 24 + (r + 1) * 8],
                            in_max=v8[:], in_values=D[:])
                        if r < 2:
                            nc.vector.match_replace(out=D[:], in_to_replace=v8[:],
                                                    in_values=D[:], imm_value=-3.0e38)
                IDXs.append(IDX)

            # index lists -> wrapped int16 idxs tile, via 16-wide PE transposes
            idxs_sb = pw.tile([128, WROW], I16, tag="idxs", name="idxs")
            for s in range(SPC):
                idxF = pw.tile([128, T * 32], F32, tag="idxF", name="idxF")
                nc.vector.memset(idxF[:], 0.0)
                i_v = IDXs[s][:].rearrange("p (t j) -> p t j", t=T, j=24)
                o_v = idxF[:].rearrange("p (t b a) -> p t b a", t=T, b=2, a=16)
                nc.vector.tensor_copy(out=o_v[:, :, 0:1, 0:16],
                                      in_=i_v[:, :, 0:16].unsqueeze(2))
                nc.vector.tensor_copy(out=o_v[:, :, 1:2, 0:4],
                                      in_=i_v[:, :, 16:20].unsqueeze(2))
                for t in range(T):
                    for b_ in range(2):
                        pst = psA.tile([16, 128], F32, tag="a", name="pst")
                        nc.tensor.transpose(
                            out=pst[:],
                            in_=idxF[:, t * 32 + b_ * 16: t * 32 + b_ * 16 + 16],
                            identity=eye_sb[:])
                        dst = AP(idxs_sb.tensor,
                                 idxs_sb.offset + (s * 64) * WROW + t * 256 + b_,
                                 [[WROW, 16], [2, 128]])
                        nc.scalar.activation(out=dst, in_=pst[:], func=AF.Copy)
                for g in (1, 2, 3):
                    nc.sync.dma_start(
                        out=idxs_sb[s * 64 + g * 16: s * 64 + (g + 1) * 16, :],
                        in_=idxs_sb[s * 64: s * 64 + 16, :])

            # gather + h1 + conv2 + max over k, in half-tiles of 64 points
            for t in range(T):
                for h in range(2):
                    G = pG.tile([128, 64 * KP], F32, tag="G", name="G")
                    nc.gpsimd.ap_gather(
                        out_ap=G[:], in_ap=A_t[:],
                        idxs_ap=idxs_sb[:, t * 256 + h * 128: t * 256 + (h + 1) * 128],
                        channels=128, num_elems=N, d=1, num_idxs=64 * KP)
                    pcol = t * 128 + h * 64
                    Gv = G[:].rearrange("c (p j) -> c p j", p=64, j=KP)[:, :, 0:K]
                    Pb = P_t[:, pcol:pcol + 64].unsqueeze(2).to_broadcast([128, 64, K])
                    nc.vector.tensor_tensor(out=Gv, in0=Gv, in1=Pb, op=OP.add)
                    nc.scalar.activation(out=Gv, in_=Gv, func=AF.Relu)

                    def conv2_pool(lhsT_sb_ap, rows):
                        psc = psC.tile([128, 64 * K], F32, tag="c", name="psc")
                        rhs_all = G[rows[0]:rows[1], :].rearrange(
                            "c (p j) -> c j p", p=64, j=KP)
                        for (j0, j1) in JBS:
                            nc.tensor.matmul(out=psc[:, j0 * 64:j1 * 64],
                                             lhsT=lhsT_sb_ap,
                                             rhs=rhs_all[:, j0:j1, :],
                                             start=True, stop=True)
                        red = small.tile([128, 64], F32, tag="red", name="red")
                        nc.vector.tensor_reduce(
                            out=red[:],
                            in_=psc[:, 0:64 * K].rearrange("c (j p) -> c p j",
                                                           j=K, p=64),
                            axis=AX.X, op=OP.max)
                        return red

                    if k == 0:
                        for s in range(SPC):
                            red = conv2_pool(c2T0_sb[s * 64:(s + 1) * 64, :],
                                             (s * 64, (s + 1) * 64))
                            nc.scalar.activation(out=X12[s][:, pcol:pcol + 64],
                                                 in_=red[:], func=AF.Relu,
                                                 bias=c2b0_sb[:])
                    else:
                        red = conv2_pool(blkT_sb[k][:], (0, 128))
                        for s in range(SPC):
                            rslice = red[s * 64:(s + 1) * 64, :]
                            bias = blkb_sb[k][s * 64:(s + 1) * 64, :]
                            if k == 1:
                                dst = X12[s][0:64, pcol:pcol + 64]
                            elif k == 2:
                                dst = X12[s][64:128, pcol:pcol + 64]
                            else:
                                dst = X3[s * 64:(s + 1) * 64, pcol:pcol + 64]
                            nc.scalar.activation(out=dst, in_=rslice, func=AF.Relu,
                                                 bias=bias)

        def vec_roundtrip(s, y_ap, nwords, ncols, rtoff):
            """(1, nwords) sbuf -> dram rt -> (128, ncols) sbuf (column-major)."""
            nc.sync.dma_start(out=rt[s:s + 1, rtoff:rtoff + nwords], in_=y_ap)
            gv = small.tile([128, max(1, ncols)], F32, tag="gv", name="gv")
            rbase = rt[s]
            nc.sync.dma_start(out=gv[:, 0:ncols],
                              in_=AP(rbase.tensor, rbase.offset + rtoff,
                                     [[1, 128], [128, ncols]]))
            return gv

        def _fc_tail(ps, M, b_sb, relu, name):
            y = small.tile([1, max(16, M)], F32, tag="fcy", name=name)
            if b_sb is None:
                nc.scalar.activation(out=y[:, 0:M], in_=ps[:], func=AF.Copy)
            else:
                nc.vector.tensor_tensor(out=y[:, 0:M], in0=ps[:], in1=b_sb[:],
                                        op=OP.add)
            if relu:
                nc.vector.tensor_scalar_max(y[:, 0:M], y[:, 0:M], 0.0)
            return y

        def fc(s, gv_sb, wT_sb, nchunk, M, b_sb, relu):
            """resident weights: out = act(W @ g + b)."""
            ps = psA.tile([1, M], F32, tag="a", name="psfc")
            for c in range(nchunk):
                nc.tensor.matmul(out=ps[:], lhsT=gv_sb[:, c:c + 1],
                                 rhs=wT_sb[:, c * M:(c + 1) * M],
                                 start=(c == 0), stop=(c == nchunk - 1))
            return _fc_tail(ps, M, b_sb, relu, "fcy")

        def fc_stream(s, gv_sb, wT_dram, nchunk, M, b_sb, relu):
            """streamed weights: chunks loaded from DRAM on the fly."""
            ps = psA.tile([1, M], F32, tag="a", name="psfs")
            for c in range(nchunk):
                wt = pw.tile([128, M], F32, tag="fcw", bufs=2, name="fcw")
                nc.sync.dma_start(out=wt[:], in_=wT_dram[c * 128:(c + 1) * 128, :])
                nc.tensor.matmul(out=ps[:], lhsT=gv_sb[:, c:c + 1],
                                 rhs=wt[:],
                                 start=(c == 0), stop=(c == nchunk - 1))
            return _fc_tail(ps, M, b_sb, relu, "fcy2")

        # ================= TNet =================
        edge_unit(0, augL, augR)
        for s in range(SPC):
            gtn = small.tile([128, 8], F32, tag=f"gtn{s}", name=f"gtn{s}")
            for oc in range(8):
                psl = psC.tile([128, N], F32, tag="c", name="psl")
                for q in range(NCB):
                    nc.tensor.matmul(out=psl[:, q * CB:(q + 1) * CB],
                                     lhsT=tn_locT_sb[:, oc * 128:(oc + 1) * 128],
                                     rhs=X12[s][:, q * CB:(q + 1) * CB],
                                     start=True, stop=True)
                nc.vector.tensor_reduce(out=gtn[:, oc:oc + 1], in_=psl[:, 0:N],
                                        axis=AX.X, op=OP.max)
            nc.vector.tensor_tensor(out=gtn[:], in0=gtn[:], in1=tn_loc_b_sb[:], op=OP.add)
            nc.vector.tensor_scalar_max(gtn[:], gtn[:], 0.0)

            y0 = fc_stream(s, gtn, tn_g0T, 8, 512, tn_g0_b_sb, True)
            gv1 = vec_roundtrip(s, y0[:, 0:512], 512, 4, 0)
            y1 = fc(s, gv1, tn_g1T_sb, 4, 256, tn_g1_b_sb, True)
            gv2 = vec_roundtrip(s, y1[:, 0:256], 256, 2, 512)
            y2 = fc(s, gv2, tn_linT_sb, 2, 9, tn_lin_be_sb, False)
            nc.sync.dma_start(out=trans_out[s:s + 1, :], in_=y2[:, 0:9])
            trT_sb = small.tile([3, 3], F32, tag="trT", name="trT")
            tob = trans_out[s]
            nc.sync.dma_start(out=trT_sb[:],
                              in_=AP(tob.tensor, tob.offset, [[1, 3], [3, 3]]))
            # x_trans -> unit-1 aug (re-load raw points from DRAM)
            ptk = pw.tile([3, N], F32, tag="ptk", name="ptk")
            nc.sync.dma_start(out=ptk[:], in_=pts[s])
            aL, aR = new_aug(s, 3)
            for q in range(NCB):
                ps = psA.tile([3, CB], F32, tag="a", name="psx")
                nc.tensor.matmul(out=ps[:], lhsT=trT_sb[:],
                                 rhs=ptk[:, q * CB:(q + 1) * CB],
                                 start=True, stop=True)
                nc.scalar.activation(out=aR[0:3, q * CB:(q + 1) * CB], in_=ps[:],
                                     func=AF.Copy)
                nc.scalar.activation(out=aL[0:3, q * CB:(q + 1) * CB], in_=ps[:],
                                     func=AF.Copy)
            fill_sq(aR, aR[0:3, :], 3)
            augL[s], augR[s] = aL, aR

        # ================= EC0/EC1/EC2 =================
        for k in (1, 2, 3):
            edge_unit(k, augL, augR)
            if k == 3:
                break
            for s in range(SPC):
                src = X12[s][0:64, :] if k == 1 else X12[s][64:128, :]
                aL, aR = new_aug(s, 64)
                nc.scalar.activation(out=aL[0:64, :], in_=src, func=AF.Copy)
                nc.scalar.activation(out=aR[0:64, :], in_=src, func=AF.Copy)
                fill_sq(aR, aR[0:64, :], 64)
                augL[s], augR[s] = aL, aR

        # ================= heads =================
        for s in range(SPC):
            # loc -> gfeat (128, 8)
            gf = small.tile([128, 8], F32, tag=f"gf{s}", name=f"gf{s}")
            for oc in range(8):
                psl = psC.tile([128, N], F32, tag="c", name="psl2")
                for q in range(NCB):
                    nc.tensor.matmul(out=psl[:, q * CB:(q + 1) * CB],
                                     lhsT=locTa_sb[:, oc * 128:(oc + 1) * 128],
                                     rhs=X12[s][:, q * CB:(q + 1) * CB],
                                     start=True, stop=False)
                    nc.tensor.matmul(out=psl[:, q * CB:(q + 1) * CB],
                                     lhsT=locTb_sb[s * 64:(s + 1) * 64, oc * 128:(oc + 1) * 128],
                                     rhs=X3[s * 64:(s + 1) * 64, q * CB:(q + 1) * CB],
                                     start=False, stop=True)
                nc.vector.tensor_reduce(out=gf[:, oc:oc + 1], in_=psl[:, 0:N],
                                        axis=AX.X, op=OP.max)
            nc.vector.tensor_tensor(out=gf[:], in0=gf[:], in1=loc_b_sb[:], op=OP.add)
            nc.vector.tensor_scalar_max(gf[:], gf[:], 0.0)

            # global branch
            ya = fc_stream(s, gf, g0T, 8, 256, g0_b_sb, True)
            gva = vec_roundtrip(s, ya[:, 0:256], 256, 2, 0)
            yb = fc(s, gva, g1T_sb, 2, 256, g1_b_sb, True)
            gvb = vec_roundtrip(s, yb[:, 0:256], 256, 2, 256)
            yc = fc(s, gvb, g2T_sb, 2, 128, g2_b_sb, True)
            gvc = vec_roundtrip(s, yc[:, 0:128], 128, 1, 512)
            yd = fc(s, gvc, goT_sb, 1, 16, go_b_sb, False)
            nc.sync.dma_start(out=global_out[s:s + 1, :], in_=yd[:, 0:16])

            # seg0 column bias = seg0_w[:, :1024] @ gfeat + seg0_b -> (128, 2)
            yq = fc_stream(s, gf, seg0AT, 8, 256, None, False)
            qcol = vec_roundtrip(s, yq[:, 0:256], 256, 2, 768)
            qb = small.tile([128, 2], F32, tag=f"qb{s}", name=f"qb{s}")
            nc.vector.tensor_tensor(out=qb[:], in0=qcol[:, 0:2], in1=seg0_bp_sb[:],
                                    op=OP.add)

            # seg stack, processed in CB-wide column chunks
            for q in range(NCB):
                qs = slice(q * CB, (q + 1) * CB)
                s0t = []
                for oc in range(2):
                    psl = psA.tile([128, CB], F32, tag="a", name="pss0")
                    nc.tensor.matmul(out=psl[:],
                                     lhsT=seg0BTa_sb[:, oc * 128:(oc + 1) * 128],
                                     rhs=X12[s][:, qs], start=True, stop=False)
                    nc.tensor.matmul(out=psl[:],
                                     lhsT=seg0BTb_sb[s * 64:(s + 1) * 64, oc * 128:(oc + 1) * 128],
                                     rhs=X3[s * 64:(s + 1) * 64, qs],
                                     start=False, stop=True)
                    st = pH.tile([128, CB], F32, tag=f"sa{oc}", name=f"sa{oc}")
                    nc.scalar.activation(out=st[:], in_=psl[:], func=AF.Relu,
                                         bias=qb[:, oc:oc + 1])
                    s0t.append(st)
                s1t = []
                for oc in range(2):
                    psl = psA.tile([128, CB], F32, tag="a", name="pss1")
                    for kc in range(2):
                        nc.tensor.matmul(
                            out=psl[:],
                            lhsT=seg1T_sb[:, (kc * 2 + oc) * 128:(kc * 2 + oc + 1) * 128],
                            rhs=s0t[kc][:], start=(kc == 0), stop=(kc == 1))
                    st = pH.tile([128, CB], F32, tag=f"sb{oc}", name=f"sb{oc}")
                    nc.scalar.activation(out=st[:], in_=psl[:], func=AF.Relu,
                                         bias=seg1_bp_sb[:, oc:oc + 1])
                    s1t.append(st)
                psl = psA.tile([128, CB], F32, tag="a", name="psct")
                for kc in range(2):
                    nc.tensor.matmul(out=psl[:],
                                     lhsT=csT_sb[:, kc * 128:(kc + 1) * 128],
                                     rhs=s1t[kc][:], start=(kc == 0), stop=(kc == 1))
                ct = pH.tile([128, CB], F32, tag="ct", name="ct")
                nc.scalar.activation(out=ct[:], in_=psl[:], func=AF.Relu,
                                     bias=cs_bp_sb[:])
                psm = psA.tile([128, CB], F32, tag="a", name="psmo")
                nc.tensor.matmul(out=psm[0:50, :], lhsT=moT_sb[:], rhs=ct[:],
                                 start=True, stop=True)
                mo_t = pH.tile([50, CB], F32, tag="mo", name="mo_t")
                nc.vector.tensor_scalar(out=mo_t[:], in0=psm[0:50, :],
                                        scalar1=mo_b_sb[:], scalar2=None, op0=OP.add)
                nc.sync.dma_start(out=mask_out[s][:, qs], in_=mo_t[:])

    nc.compile()
    _BUILD_CACHE[N] = nc
    return nc


def host_weights(inputs):
    """Derive the device weight layouts from the raw parameter dict."""
    f = np.float32
    N = inputs["points"].shape[2]
    out = {}

    def wawp(w, b):
        C = w.shape[1] // 2
        W1, W2 = w[:, :C], w[:, C:]
        waT = np.vstack([W2.T.astype(f), np.zeros((1, 64), f)])
        wpT = np.vstack([(W1 - W2).T.astype(f), -b[None, :].astype(f)])
        return np.ascontiguousarray(waT), np.ascontiguousarray(wpT)

    for k, (wn, bn) in enumerate([("tn_ec0_w", "tn_ec0_b"), ("ec0a_w", "ec0a_b"),
                                  ("ec1a_w", "ec1a_b"), ("ec2a_w", "ec2a_b")]):
        out[f"waT{k}"], out[f"wpT{k}"] = wawp(inputs[wn], inputs[bn])
    c2t = inputs["tn_ec1_w"].T.astype(f)
    out["c2T0"] = np.ascontiguousarray(np.vstack([c2t, c2t]))
    out["c2b0"] = inputs["tn_ec1_b"].astype(f)[:, None]
    for k, (wn, bn) in zip((1, 2, 3), [("ec0b_w", "ec0b_b"), ("ec1b_w", "ec1b_b"),
                                       ("ec2b_w", "ec2b_b")]):
        w = inputs[wn].astype(f)
        blk = np.zeros((128, 128), f)
        blk[0:64, 0:64] = w.T
        blk[64:128, 64:128] = w.T
        out[f"blkT{k}"] = blk
        out[f"blkb{k}"] = np.concatenate([inputs[bn], inputs[bn]]).astype(f)[:, None]
    out["ones"] = np.ones((64, 1), f)
    out["eye128"] = np.eye(128, dtype=f)
    out["neg1"] = np.full((1, N), -1.0, f)
    out["tn_locT"] = np.ascontiguousarray(inputs["tn_loc_w"].T.astype(f))
    out["tn_loc_b"] = np.ascontiguousarray(inputs["tn_loc_b"].reshape(8, 128).T.astype(f))
    out["tn_g0T"] = np.ascontiguousarray(inputs["tn_g0_w"].T.astype(f))
    out["tn_g0_b"] = inputs["tn_g0_b"].astype(f)[None, :]
    out["tn_g1T"] = np.ascontiguousarray(inputs["tn_g1_w"].T.astype(f))
    out["tn_g1_b"] = inputs["tn_g1_b"].astype(f)[None, :]
    out["tn_linT"] = np.ascontiguousarray(inputs["tn_lin_w"].T.astype(f))
    out["tn_lin_be"] = (inputs["tn_lin_b"].astype(f)
                        + np.eye(3, dtype=f).ravel())[None, :]
    out["locTa"] = np.ascontiguousarray(inputs["loc_w"][:, 0:128].T.astype(f))
    ltb = inputs["loc_w"][:, 128:192].T.astype(f)
    out["locTb"] = np.ascontiguousarray(np.vstack([ltb, ltb]))
    out["loc_b"] = np.ascontiguousarray(inputs["loc_b"].reshape(8, 128).T.astype(f))
    out["g0T"] = np.ascontiguousarray(inputs["g0_w"].T.astype(f))
    out["g0_b"] = inputs["g0_b"].astype(f)[None, :]
    out["g1T"] = np.ascontiguousarray(inputs["g1_w"].T.astype(f))
    out["g1_b"] = inputs["g1_b"].astype(f)[None, :]
    out["g2T"] = np.ascontiguousarray(inputs["g2_w"].T.astype(f))
    out["g2_b"] = inputs["g2_b"].astype(f)[None, :]
    out["goT"] = np.ascontiguousarray(inputs["go_w"].T.astype(f))
    out["go_b"] = inputs["go_b"].astype(f)[None, :]
    out["seg0AT"] = np.ascontiguousarray(inputs["seg0_w"][:, 0:1024].T.astype(f))
    out["seg0BTa"] = np.ascontiguousarray(inputs["seg0_w"][:, 1024:1152].T.astype(f))
    s0b = inputs["seg0_w"][:, 1152:1216].T.astype(f)
    out["seg0BTb"] = np.ascontiguousarray(np.vstack([s0b, s0b]))
    out["seg0_bp"] = np.ascontiguousarray(inputs["seg0_b"].reshape(2, 128).T.astype(f))
    out["seg1T"] = np.ascontiguousarray(inputs["seg1_w"].T.astype(f))
    out["seg1_bp"] = np.ascontiguousarray(inputs["seg1_b"].reshape(2, 128).T.astype(f))
    out["csT"] = np.ascontiguousarray(inputs["cs_w"].T.astype(f))
    out["cs_bp"] = inputs["cs_b"].astype(f)[:, None].reshape(128, 1)
    out["moT"] = np.ascontiguousarray(inputs["mo_w"].T.astype(f))
    out["mo_b"] = inputs["mo_b"].astype(f)[:, None]
    return out


LAST_WALL_S = None


def kernel(**inputs):
    import concourse.bass_utils as bass_utils
    import time

    N = inputs["points"].shape[2]
    nc = _build(N)
    shared = host_weights(inputs)
    in_maps = []
    for c in range(NCORES):
        m = dict(shared)
        m["pts"] = np.ascontiguousarray(
            inputs["points"][c * SPC:(c + 1) * SPC].astype(np.float32))
        in_maps.append(m)

    t0 = time.perf_counter()
    res = bass_utils.run_bass_kernel_spmd(nc, in_maps, core_ids=list(range(NCORES)))
    global LAST_WALL_S
    LAST_WALL_S = time.perf_counter() - t0

    g = np.concatenate([res.results[c]["global_out"] for c in range(NCORES)], axis=0)
    m = np.concatenate([res.results[c]["mask_out"] for c in range(NCORES)], axis=0)
    tr = np.concatenate([res.results[c]["trans_out"] for c in range(NCORES)],
                        axis=0).reshape(B, 3, 3)
    return g, m, tr
